# revision 35
# baseline (speedup 1.0000x reference)
"""
Trainium2 Bass kernel for nn_Encoder (embedding lookup + LSTM, returns final (h, c)).

Strategy (data-parallel over batch, per sharding hint):
  - 8 cores, each handles B_local = 4 of the 32 batch rows.
  - Per core: gather embedding rows via indirect DMA (t-major order),
    transpose on PE, project x @ W with fp16 matmuls (chunked over T),
    then run the 512-step recurrence with U as the stationary operand
    in fp16 producing gates transposed (4H on partitions) so
    activations/cell update run wide on ACT/DVE.
  - Gate layout: psum z tile per pair of H-slices, packed columns
    (gate', b) with gate' order (i, f, o, g) so one sigmoid covers i,f,o
    and one tanh covers g.
  - h is kept as hT [128 x (hs, b)] fp16 which is exactly the moving-operand
    layout the next step's matmuls need.
  - Per-step MM blocks follow ORDER625 (see emit_step_v5): interleaving the
    two psum groups' k-rounds maximizes the slack available to each pair's
    cross-engine activation/update chain.

Performance notes (measured on HW, this session): the recurrence runs at
~64 x ~27ns LDWEIGHTS+MATMUL pairs per step and is pinned there — fp8
weights (e3m4, FWL) change nothing (pace is pair-overhead/weight-load-
floor bound, not weight-byte bound), and folding the LDW into a
self-loading matmul (InstMatmult.ldweights=True) produces NaNs on HW
(walrus needs the standalone LDW for non-fp32). Alternative schedules
(xz injection into PSUM via identity-matmul, 4-way group pipelining,
prep spread into step bubbles) all measure within noise (+-80ns/step).

Host side: shard/marshal inputs, run SPMD on 8 cores, unpack outputs.
"""

import numpy as np

import concourse.bass as bass
import concourse.mybir as mybir
import concourse.tile as tile
from concourse import bacc
from concourse.bass import IndirectOffsetOnAxis
from concourse.bass_utils import run_bass_kernel_spmd
from concourse.masks import make_identity

# Problem constants (hardcoded; harness contract)
B, T, V, E, H = 32, 512, 20000, 300, 512
G4 = 4 * H            # 2048
NCORES = 8
BL = B // NCORES      # 4 batch rows per core
P = 128
KM = G4 // P          # 16 M-tiles over 4H
KH = H // P           # 4 K-tiles over H
KE_SIZES = [128, 128, 44]   # K subtiles over E=300
# Keras gate g (i,f,g,o) -> packed slot (i,f,o,g): sigmoid = slots 0..2, tanh = slot 3
PERM = [0, 1, 3, 2]

f32 = mybir.dt.float32
f32r = mybir.dt.float32r
f16 = mybir.dt.float16
f8e3 = mybir.dt.float8e3
i32 = mybir.dt.int32

AF = mybir.ActivationFunctionType


def build_program(
    nc, T_steps=T, Tc=128, dbg_step=None, reps=1, sched="v2", u8=True, spread=False
):
    """Emit the full per-core program into nc (a bacc.Bacc).

    reps > 1 repeats the whole compute (for timing amplification).

    u8=True stores the recurrent weights U in fp8 (e3m4) scaled by USCALE,
    with the xz path (W, b) carrying the same scale and the activations
    descaling via their input-scale operand. Measured: identical speed to
    fp16 (the per-matmul pace is bound by the LDWEIGHTS+MATMUL pair
    overhead, not weight bytes), ~10x the numeric error — so keep False.

    sched picks the per-step schedule: v2 = pair-contiguous MM order with
    DVE z-add; v3/v4 = xz injected into PSUM via identity-matmul with a
    shortened chain; v5 = v2's chain with an interleaved MM block order
    (ORDER625) that raises both pairs' chain slack from 0.25*S to 0.375*S.
    All measured within noise of each other on hardware (the 512-step
    recurrence is pinned at ~64 LDW+MM pairs x ~27ns/step).

    spread=True (v3/v4/v5) breaks the next chunk's gather/transpose/xz-
    projection into single-op thunks slotted into per-step PE bubbles
    instead of one serial burst."""
    assert T_steps % Tc == 0
    USCALE = 32.0 if u8 else 1.0
    INV = 1.0 / USCALE
    udt = f8e3 if u8 else f16
    nch = T_steps // Tc
    NJ = Tc * BL // P  # gathers (128-row groups) per chunk

    emb_t = nc.declare_dram_parameter("emb", [V, E], f32, isOutput=False)
    W_t = nc.declare_dram_parameter("W", [E, G4], f32, isOutput=False)
    U_t = nc.declare_dram_parameter("U", [H, G4], f32, isOutput=False)
    b_t = nc.declare_dram_parameter("bvec", [G4], f32, isOutput=False)
    tok_t = nc.declare_dram_parameter("tok", [P, T_steps * BL // P], i32, isOutput=False)
    ho_t = nc.declare_dram_parameter("ho", [P, BL * KH], f16, isOutput=True)
    co_t = nc.declare_dram_parameter("co", [P, BL * KH], f32, isOutput=True)
    if dbg_step is not None:
        dbg_z = nc.declare_dram_parameter("dbg_z", [P, 64], f32, isOutput=True)
        dbg_h = nc.declare_dram_parameter("dbg_h", [P, BL * KH], f16, isOutput=True)
        dbg_c = nc.declare_dram_parameter("dbg_c", [P, BL * KH], f32, isOutput=True)

    with tile.TileContext(nc) as tc:
        with (
            tc.tile_pool(name="const", bufs=1) as cpool,
            tc.tile_pool(name="ustage", bufs=2) as upool,
            tc.tile_pool(name="xrows", bufs=4) as xpool,
            tc.tile_pool(name="xtp", bufs=2) as xtpool,
            tc.tile_pool(name="ptr", bufs=2, space="PSUM") as ptr_pool,
            tc.tile_pool(name="pxz", bufs=2, space="PSUM") as pxz_pool,
            tc.tile_pool(name="pz", bufs=4, space="PSUM") as pz_pool,
        ):
            # ---- constants / weights ----
            U16 = cpool.tile([P, KH * G4], udt, tag="U16")
            W_sb = cpool.tile([P, 3 * G4], f16, tag="Wsb")
            b_sb = cpool.tile([P, KM], f32, tag="bsb")
            tok_sb = cpool.tile([P, T_steps * BL // P], i32, tag="tok")
            ident = cpool.tile([P, P], f32, tag="ident")
            h16 = cpool.tile([P, BL * KH], f16, tag="h16")
            cst = cpool.tile([P, BL * KH], f32, tag="cst")
            z_s = cpool.tile([P, 64], f32, tag="zs")
            a_s = cpool.tile([P, 64], f32, tag="as")
            tmp1 = cpool.tile([P, BL * KH], f32, tag="t1")
            tmp2 = cpool.tile([P, BL * KH], f32, tag="t2")
            tct = cpool.tile([P, BL * KH], f32, tag="tct")
            xzdt = f16 if sched in ("v3", "v4") else f32
            xz_sb = [
                cpool.tile([P, Tc * 64], xzdt, tag=f"xz{par}", name=f"xz{par}")
                for par in range(2)
            ]

            make_identity(nc, ident[:])

            # U (fp32 DRAM) -> U16 (fp16 SBUF), K-tile k region at cols k*G4
            for k in range(KH):
                ust = upool.tile([P, G4], f32, tag="ustage")
                nc.sync.dma_start(ust[:], U_t.ap()[k * P:(k + 1) * P, :])
                if u8:
                    nc.vector.tensor_scalar_mul(
                        U16[:, k * G4:(k + 1) * G4], ust[:], USCALE
                    )
                else:
                    nc.vector.tensor_copy(U16[:, k * G4:(k + 1) * G4], ust[:])

            # W: 3 K-subtiles at cols kk*G4, cast to fp16 via staging
            ofs = 0
            for kk, kw in enumerate(KE_SIZES):
                wst = upool.tile([P, G4], f32, tag="ustage", name=f"wst{kk}")
                nc.sync.dma_start(wst[:kw, :], W_t.ap()[ofs:ofs + kw, :])
                if u8:
                    nc.vector.tensor_scalar_mul(
                        W_sb[:kw, kk * G4:(kk + 1) * G4], wst[:kw, :], USCALE
                    )
                else:
                    nc.vector.tensor_copy(
                        W_sb[:kw, kk * G4:(kk + 1) * G4], wst[:kw, :]
                    )
                ofs += kw

            # bias: b_sb[p, m] = b[m*128 + p]
            nc.sync.dma_start(b_sb[:], b_t.ap().rearrange("(m p) -> p m", p=P))
            if u8:
                nc.vector.tensor_scalar_mul(b_sb[:], b_sb[:], USCALE)
            nc.sync.dma_start(tok_sb[:], tok_t.ap())

            nc.gpsimd.memset(h16[:], 0.0)
            nc.gpsimd.memset(cst[:], 0.0)

            def make_prep_thunks(c):
                """Prep for chunk c as a list of single-PE-op thunks.

                thunks[0] issues the gather DMAs (off-PE); the rest each emit
                one PE op (transpose or one xz matmul K-subtile) so they can
                be slotted into per-step PE wait bubbles."""
                xz_dst = xz_sb[c % 2]
                xT = xtpool.tile([P, 3 * Tc * BL], f16, tag="xT", name=f"xT{c}")
                xrs = []
                state = {}

                def gathers():
                    for j in range(NJ):
                        xr = xpool.tile([P, E], f32, tag="xrows", name=f"xr{c}_{j}")
                        xrs.append(xr)
                        nc.gpsimd.indirect_dma_start(
                            out=xr[:],
                            out_offset=None,
                            in_=emb_t.ap(),
                            in_offset=IndirectOffsetOnAxis(
                                ap=tok_sb[:, c * NJ + j:c * NJ + j + 1], axis=0
                            ),
                        )

                thunks = [gathers]

                def tp(j, kk, kw):
                    def run():
                        pt = ptr_pool.tile([P, P], f32, tag="ptr")
                        nc.tensor.transpose(
                            out=pt[:kw, :], in_=xrs[j][:, kk * P:kk * P + kw],
                            identity=ident[:],
                        )
                        nc.vector.tensor_copy(
                            xT[:kw, kk * Tc * BL + j * P:kk * Tc * BL + (j + 1) * P],
                            pt[:kw, :],
                        )
                    return run

                for j in range(NJ):
                    for kk, kw in enumerate(KE_SIZES):
                        thunks.append(tp(j, kk, kw))

                N = Tc * BL

                def mm(m, kk, kw):
                    def run():
                        if kk == 0:
                            state["pxz"] = pxz_pool.tile(
                                [P, N], f32, tag="pxz", name=f"pxz{c}_{m}"
                            )
                        pxz = state["pxz"]
                        nc.tensor.matmul(
                            pxz[:],
                            W_sb[:kw, kk * G4 + m * P:kk * G4 + (m + 1) * P],
                            xT[:kw, kk * N:(kk + 1) * N],
                            start=(kk == 0),
                            stop=(kk == 2),
                        )
                        if kk == 2:
                            # packed dest: col = t*64 + (m%4)*16 + PERM[m//4]*4 + b
                            slot = (m % 4) * 16 + PERM[m // 4] * 4
                            dst = xz_dst[:].rearrange("p (t g) -> p t g", g=64)[
                                :, :, slot:slot + 4
                            ]
                            src = pxz[:].rearrange("p (t b) -> p t b", b=BL)
                            nc.vector.tensor_scalar_add(dst, src, b_sb[:, m:m + 1])
                    return run

                for m in range(KM):
                    for kk, kw in enumerate(KE_SIZES):
                        thunks.append(mm(m, kk, kw))
                return thunks

            def emit_prep(c):
                """Gather + transpose + xz projection for chunk c, all at once."""
                for th in make_prep_thunks(c):
                    th()

            # v3 state/scratch: one tile, 24 cols per hs: [i f o g | c | tct]
            asc = cpool.tile([P, 4 * 24], f32, tag="asc")
            tmp3 = cpool.tile([P, 4 * 8], f32, tag="tmp3")
            ident8 = cpool.tile([P, P], udt, tag="ident8")
            nc.vector.tensor_copy(ident8[:], ident[:])
            nc.gpsimd.memset(asc[:], 0.0)

            # MM emission order for the last K round: group M-tiles by H-slice
            ORDER_LAST = [m for hs in range(4) for m in (hs, 4 + hs, 8 + hs, 12 + hs)]

            def emit_step_v1(c, t):
                psz = [
                    pz_pool.tile([P, 16], f32, tag="pz", name=f"pz{hs}_{c}_{t}")
                    for hs in range(4)
                ]
                for k in range(KH):
                    order = ORDER_LAST if k == KH - 1 else range(KM)
                    for m in order:
                        slot = PERM[m // 4] * 4
                        # start=True marks the whole 2KB psum bank pending-zero,
                        # so only the FIRST matmul touching each psz tile sets it
                        # (round k=0, m in 0..3); later slots overwrite via
                        # pending-zero, later k rounds accumulate.
                        nc.tensor.matmul(
                            psz[m % 4][:, slot:slot + 4],
                            U16[:, k * G4 + m * P:k * G4 + (m + 1) * P],
                            h16[:, k * BL:(k + 1) * BL],
                            start=(k == 0 and m < 4),
                            stop=(k == KH - 1),
                            skip_group_check=True,
                        )
                for hs in range(4):
                    zs = z_s[:, hs * 16:hs * 16 + 16]
                    nc.vector.tensor_add(
                        zs,
                        psz[hs][:],
                        xz_sb[c % 2][:, t * 64 + hs * 16:t * 64 + hs * 16 + 16],
                    )
                    # sigmoid over (i, f, o) slots, tanh over g slot
                    nc.scalar.activation(
                        a_s[:, hs * 16:hs * 16 + 12], z_s[:, hs * 16:hs * 16 + 12],
                        AF.Sigmoid, scale=INV,
                    )
                    nc.scalar.activation(
                        a_s[:, hs * 16 + 12:hs * 16 + 16],
                        z_s[:, hs * 16 + 12:hs * 16 + 16],
                        AF.Tanh, scale=INV,
                    )
                    cs = slice(hs * BL, (hs + 1) * BL)
                    nc.vector.tensor_mul(
                        tmp1[:, cs], a_s[:, hs * 16 + 4:hs * 16 + 8], cst[:, cs]
                    )  # f * c
                    nc.vector.tensor_mul(
                        tmp2[:, cs],
                        a_s[:, hs * 16:hs * 16 + 4],
                        a_s[:, hs * 16 + 12:hs * 16 + 16],
                    )  # i * g
                    nc.vector.tensor_add(cst[:, cs], tmp1[:, cs], tmp2[:, cs])
                    nc.scalar.activation(tct[:, cs], cst[:, cs], AF.Tanh)
                    nc.vector.tensor_mul(
                        h16[:, cs], a_s[:, hs * 16 + 8:hs * 16 + 12], tct[:, cs]
                    )  # h = o * tanh(c), cast to fp16 on write

            def a2(base, width):
                """2D AP over a_s/z_s: [128, (2 hs, width)] at col base within
                each 16-col hs block of the pair being processed."""
                return base.rearrange("p (hs w) -> p hs w", w=16)

            def emit_step_v2(c, t):
                # 2 psum tiles, one per hs-pair; cols = (hs%2)*16 + slot*4 + b
                psz = [
                    pz_pool.tile([P, 32], f32, tag="pz", name=f"pzp{pr}_{c}_{t}")
                    for pr in range(2)
                ]
                # pair-major PE order: all of pair 0's MMs (k-outer), then pair 1
                for pr in range(2):
                    for k in range(KH):
                        for hs in (2 * pr, 2 * pr + 1):
                            for g in range(4):
                                m = g * 4 + hs
                                slot = (hs % 2) * 16 + PERM[g] * 4
                                nc.tensor.matmul(
                                    psz[pr][:, slot:slot + 4],
                                    U16[:, k * G4 + m * P:k * G4 + (m + 1) * P],
                                    h16[:, k * BL:(k + 1) * BL],
                                    start=(k == 0 and hs == 2 * pr and g == 0),
                                    stop=(k == KH - 1),
                                    skip_group_check=True,
                                )
                xz = xz_sb[c % 2]
                for pr in range(2):
                    # per-hs adds (start as soon as that hs' slots are done)
                    for hs in (2 * pr, 2 * pr + 1):
                        nc.vector.tensor_add(
                            z_s[:, hs * 16:hs * 16 + 16],
                            psz[pr][:, (hs % 2) * 16:(hs % 2) * 16 + 16],
                            xz[:, t * 64 + hs * 16:t * 64 + hs * 16 + 16],
                        )
                    h0 = 2 * pr * 16  # base col of this pair in z_s/a_s
                    zs2 = z_s[:].rearrange("p (hs w) -> p hs w", w=16)
                    as2 = a_s[:].rearrange("p (hs w) -> p hs w", w=16)
                    # sigmoid over (i,f,o) of both hs in one 2D-AP instr
                    nc.scalar.activation(
                        as2[:, 2 * pr:2 * pr + 2, 0:12],
                        zs2[:, 2 * pr:2 * pr + 2, 0:12],
                        AF.Sigmoid, scale=INV,
                    )
                    nc.scalar.activation(
                        as2[:, 2 * pr:2 * pr + 2, 12:16],
                        zs2[:, 2 * pr:2 * pr + 2, 12:16],
                        AF.Tanh, scale=INV,
                    )
                    cs = slice(pr * 2 * BL, (pr + 1) * 2 * BL)  # 8 cols of cst
                    c2 = cst[:, cs].rearrange("p (hs b) -> p hs b", b=BL)
                    t1 = tmp1[:, cs].rearrange("p (hs b) -> p hs b", b=BL)
                    t2 = tmp2[:, cs].rearrange("p (hs b) -> p hs b", b=BL)
                    nc.vector.tensor_mul(
                        t1, as2[:, 2 * pr:2 * pr + 2, 4:8], c2
                    )  # f * c
                    nc.vector.tensor_mul(
                        t2,
                        as2[:, 2 * pr:2 * pr + 2, 0:4],
                        as2[:, 2 * pr:2 * pr + 2, 12:16],
                    )  # i * g
                    nc.vector.tensor_add(cst[:, cs], tmp1[:, cs], tmp2[:, cs])
                    nc.scalar.activation(tct[:, cs], cst[:, cs], AF.Tanh)
                    nc.vector.tensor_mul(
                        h16[:, cs].rearrange("p (hs b) -> p hs b", b=BL),
                        as2[:, 2 * pr:2 * pr + 2, 8:12],
                        tct[:, cs].rearrange("p (hs b) -> p hs b", b=BL),
                    )  # h = o * tanh(c), cast to fp16 on write

            def emit_step_v3(c, t, thunk=None):
                """xz injected into PSUM via identity-matmul (no DVE z-add);
                activations read PSUM; per-hs packed state [i f o g | c | tct]
                with one merged mul for [i*g | f*c]. `thunk` (prep work) is
                slotted before pr0's k=2 round, where the PE waits for the
                previous step's pair-1 chain anyway."""
                xz = xz_sb[c % 2]
                psz = [
                    pz_pool.tile([P, 32], f32, tag="pz", name=f"pzv3_{pr}_{c}_{t}")
                    for pr in range(2)
                ]
                for pr in range(2):
                    nc.tensor.matmul(
                        psz[pr][:],
                        ident8[:],
                        xz[:, t * 64 + pr * 32:t * 64 + (pr + 1) * 32],
                        start=True,
                        stop=False,
                        skip_group_check=True,
                    )
                    for k in range(KH):
                        if pr == 0 and k == 2 and thunk is not None:
                            thunk()
                        for hs in (2 * pr, 2 * pr + 1):
                            for g in range(4):
                                m = g * 4 + hs
                                slot = (hs % 2) * 16 + PERM[g] * 4
                                nc.tensor.matmul(
                                    psz[pr][:, slot:slot + 4],
                                    U16[:, k * G4 + m * P:k * G4 + (m + 1) * P],
                                    h16[:, k * BL:(k + 1) * BL],
                                    start=False,
                                    stop=(k == KH - 1),
                                    skip_group_check=True,
                                )
                a3 = asc[:].rearrange("p (hs w) -> p hs w", w=24)
                t3 = tmp3[:].rearrange("p (hs w) -> p hs w", w=8)
                h3 = h16[:].rearrange("p (hs b) -> p hs b", b=BL)
                for pr in range(2):
                    ps3 = psz[pr][:].rearrange("p (hs w) -> p hs w", w=16)
                    hs0 = 2 * pr
                    sl = slice(hs0, hs0 + 2)
                    nc.scalar.activation(
                        a3[:, sl, 0:12], ps3[:, 0:2, 0:12], AF.Sigmoid, scale=INV
                    )
                    nc.scalar.activation(
                        a3[:, sl, 12:16], ps3[:, 0:2, 12:16], AF.Tanh, scale=INV
                    )
                    # [i*g | f*c] in one op: [i,f] x [g,c]
                    nc.vector.tensor_mul(
                        t3[:, sl, :], a3[:, sl, 0:8], a3[:, sl, 12:20]
                    )
                    nc.vector.tensor_add(
                        a3[:, sl, 16:20], t3[:, sl, 0:4], t3[:, sl, 4:8]
                    )
                    nc.scalar.activation(
                        a3[:, sl, 20:24], a3[:, sl, 16:20], AF.Tanh
                    )
                    nc.vector.tensor_mul(
                        h3[:, sl, :], a3[:, sl, 8:12], a3[:, sl, 20:24]
                    )

            # LP-optimal G=2 block order: balances both pairs' chain slack at
            # 0.625*S (vs 0.75*S for pair-contiguous order). (pr, k) blocks.
            ORDER625 = [(0, 0), (0, 1), (1, 0), (0, 2), (0, 3), (1, 1), (1, 2), (1, 3)]

            def emit_step_v4(c, t, thunk=None):
                """v3 chain + interleaved MM block order per ORDER625."""
                xz = xz_sb[c % 2]
                psz = [
                    pz_pool.tile([P, 32], f32, tag="pz", name=f"pzv4_{pr}_{c}_{t}")
                    for pr in range(2)
                ]
                if thunk is not None:
                    thunk()
                started = [False, False]
                for pr, k in ORDER625:
                    if not started[pr]:
                        nc.tensor.matmul(
                            psz[pr][:],
                            ident8[:],
                            xz[:, t * 64 + pr * 32:t * 64 + (pr + 1) * 32],
                            start=True,
                            stop=False,
                            skip_group_check=True,
                        )
                        started[pr] = True
                    for hs in (2 * pr, 2 * pr + 1):
                        for g in range(4):
                            m = g * 4 + hs
                            slot = (hs % 2) * 16 + PERM[g] * 4
                            nc.tensor.matmul(
                                psz[pr][:, slot:slot + 4],
                                U16[:, k * G4 + m * P:k * G4 + (m + 1) * P],
                                h16[:, k * BL:(k + 1) * BL],
                                start=False,
                                stop=(k == KH - 1),
                                skip_group_check=True,
                            )
                a3 = asc[:].rearrange("p (hs w) -> p hs w", w=24)
                t3 = tmp3[:].rearrange("p (hs w) -> p hs w", w=8)
                h3 = h16[:].rearrange("p (hs b) -> p hs b", b=BL)
                for pr in range(2):
                    ps3 = psz[pr][:].rearrange("p (hs w) -> p hs w", w=16)
                    hs0 = 2 * pr
                    sl = slice(hs0, hs0 + 2)
                    nc.scalar.activation(
                        a3[:, sl, 0:12], ps3[:, 0:2, 0:12], AF.Sigmoid, scale=INV
                    )
                    nc.scalar.activation(
                        a3[:, sl, 12:16], ps3[:, 0:2, 12:16], AF.Tanh, scale=INV
                    )
                    nc.vector.tensor_mul(
                        t3[:, sl, :], a3[:, sl, 0:8], a3[:, sl, 12:20]
                    )
                    nc.vector.tensor_add(
                        a3[:, sl, 16:20], t3[:, sl, 0:4], t3[:, sl, 4:8]
                    )
                    nc.scalar.activation(
                        a3[:, sl, 20:24], a3[:, sl, 16:20], AF.Tanh
                    )
                    nc.vector.tensor_mul(
                        h3[:, sl, :], a3[:, sl, 8:12], a3[:, sl, 20:24]
                    )

            def emit_step_v5(c, t, thunk=None):
                """v2's chain (DVE z-add, no identity-MM) with the ORDER625
                interleaved MM block order: both pairs' chain slack becomes
                0.625*S (vs 0.75*S pair-contiguous), hiding the chain fully."""
                psz = [
                    pz_pool.tile([P, 32], f32, tag="pz", name=f"pzv5_{pr}_{c}_{t}")
                    for pr in range(2)
                ]
                started = [False, False]
                if thunk is not None:
                    thunk()
                for pr, k in ORDER625:
                    first = not started[pr]
                    started[pr] = True
                    for hs in (2 * pr, 2 * pr + 1):
                        for g in range(4):
                            m = g * 4 + hs
                            slot = (hs % 2) * 16 + PERM[g] * 4
                            nc.tensor.matmul(
                                psz[pr][:, slot:slot + 4],
                                U16[:, k * G4 + m * P:k * G4 + (m + 1) * P],
                                h16[:, k * BL:(k + 1) * BL],
                                start=(first and hs == 2 * pr and g == 0),
                                stop=(k == KH - 1),
                                skip_group_check=True,
                            )
                xz = xz_sb[c % 2]
                for pr in range(2):
                    for hs in (2 * pr, 2 * pr + 1):
                        nc.vector.tensor_add(
                            z_s[:, hs * 16:hs * 16 + 16],
                            psz[pr][:, (hs % 2) * 16:(hs % 2) * 16 + 16],
                            xz[:, t * 64 + hs * 16:t * 64 + hs * 16 + 16],
                        )
                    zs2 = z_s[:].rearrange("p (hs w) -> p hs w", w=16)
                    as2 = a_s[:].rearrange("p (hs w) -> p hs w", w=16)
                    nc.scalar.activation(
                        as2[:, 2 * pr:2 * pr + 2, 0:12],
                        zs2[:, 2 * pr:2 * pr + 2, 0:12],
                        AF.Sigmoid, scale=INV,
                    )
                    nc.scalar.activation(
                        as2[:, 2 * pr:2 * pr + 2, 12:16],
                        zs2[:, 2 * pr:2 * pr + 2, 12:16],
                        AF.Tanh, scale=INV,
                    )
                    cs = slice(pr * 2 * BL, (pr + 1) * 2 * BL)
                    c2 = cst[:, cs].rearrange("p (hs b) -> p hs b", b=BL)
                    t1 = tmp1[:, cs].rearrange("p (hs b) -> p hs b", b=BL)
                    t2 = tmp2[:, cs].rearrange("p (hs b) -> p hs b", b=BL)
                    nc.vector.tensor_mul(
                        t1, as2[:, 2 * pr:2 * pr + 2, 4:8], c2
                    )
                    nc.vector.tensor_mul(
                        t2,
                        as2[:, 2 * pr:2 * pr + 2, 0:4],
                        as2[:, 2 * pr:2 * pr + 2, 12:16],
                    )
                    nc.vector.tensor_add(cst[:, cs], tmp1[:, cs], tmp2[:, cs])
                    nc.scalar.activation(tct[:, cs], cst[:, cs], AF.Tanh)
                    nc.vector.tensor_mul(
                        h16[:, cs].rearrange("p (hs b) -> p hs b", b=BL),
                        as2[:, 2 * pr:2 * pr + 2, 8:12],
                        tct[:, cs].rearrange("p (hs b) -> p hs b", b=BL),
                    )

            emit_step = {
                "v1": emit_step_v1,
                "v2": emit_step_v2,
                "v3": emit_step_v3,
                "v4": emit_step_v4,
                "v5": emit_step_v5,
            }[sched]

            for rep in range(reps):
                if rep > 0:
                    nc.gpsimd.memset(h16[:], 0.0)
                    nc.gpsimd.memset(cst[:], 0.0)
                    nc.gpsimd.memset(asc[:], 0.0)
                emit_prep(0)
                pending = []
                for c in range(nch):
                    for t in range(Tc):
                        thunk = None
                        if spread and sched in ("v3", "v4", "v5") and pending and t >= 16:
                            thunk = pending.pop(0)
                        emit_step(c, t, thunk=thunk) if sched in (
                            "v3", "v4", "v5"
                        ) else emit_step(c, t)
                        if dbg_step is not None and (c, t) == dbg_step:
                            nc.sync.dma_start(dbg_z.ap(), z_s[:])
                            nc.sync.dma_start(dbg_h.ap(), h16[:])
                            nc.sync.dma_start(dbg_c.ap(), cst[:])
                        if t == 8 and c + 1 < nch:
                            if spread and sched in ("v3", "v4", "v5"):
                                ths = make_prep_thunks(c + 1)
                                ths[0]()  # gathers go out immediately (off-PE)
                                pending = ths[1:]
                            else:
                                emit_prep(c + 1)
                    assert not pending, f"{len(pending)} prep thunks left"

            nc.sync.dma_start(ho_t.ap(), h16[:])
            if sched in ("v3", "v4"):
                nc.sync.dma_start(
                    co_t.ap(),
                    asc[:].rearrange("p (hs w) -> p hs w", w=24)[:, :, 16:20],
                )
            else:
                nc.sync.dma_start(co_t.ap(), cst[:])

    return nc


def fuse_ldweights(nc):
    """Fold each standalone InstLdweights into its paired InstMatmult
    (self-loading matmul), halving the PE instruction count. bass emits
    LDW directly before its matmul; the LDW carries the weights-tile
    dependency, which must move onto the matmul."""
    n = 0
    for f in nc.m.functions:
        for blk in f.blocks:
            pending = None
            keep = []
            for ins in blk.instructions:
                if isinstance(ins, mybir.InstLdweights):
                    assert pending is None, "two LDWs without a matmul between"
                    pending = ins
                    continue
                if (
                    isinstance(ins, mybir.InstMatmult)
                    and not (ins.is_transpose or False)
                    and pending is not None
                ):
                    ins.ldweights = True
                    ins.merge_dependencies_from(pending)
                    pending = None
                    n += 1
                keep.append(ins)
            assert pending is None, "trailing LDW without matmul"
            blk.instructions = keep
    # move_matmul_waits_to_ldweights scans backward for an LDW per multi-wait
    # matmul; with no LDWs left it degenerates to an O(n^2) full-block scan
    # (~20min at T=512) and has nothing to do anyway — skip it.
    nc.move_matmul_waits_to_ldweights = lambda: None
    return n


_CACHE = {}


# Best verified configuration for the shipped kernel() path.
# v5 = v2's chain with the LP-optimal interleaved MM order (never more
# stall than v2 at equal instruction count; measured equal-or-slightly
# faster, bit-identical numerics).
BEST = dict(sched="v5", u8=False, spread=False, fused=False)


def _get_compiled(T_steps=T, Tc=128, **kw):
    cfg = {**BEST, **kw}
    key = (T_steps, Tc, tuple(sorted(cfg.items())))
    if key not in _CACHE:
        fused = cfg.pop("fused")
        nc = bacc.Bacc(None, target_bir_lowering=False)
        build_program(nc, T_steps, Tc, **cfg)
        if fused:
            fuse_ldweights(nc)
        nc.compile()
        _CACHE[key] = nc
    return _CACHE[key]


def make_tok_idx(tokens_slice, T_steps=T):
    """tokens_slice [BL, T] -> [128, T*BL/128] int32, [p, j] = t-major flat[j*128+p]."""
    flat = tokens_slice.T.reshape(-1)  # index n = t*BL + b
    return np.ascontiguousarray(
        flat.reshape(T_steps * BL // P, P).T.astype(np.int32)
    )


def unpack_state(arr):
    """[128, 16] packed (p, hs*4+b) -> [BL, H]."""
    a = np.asarray(arr).astype(np.float32).reshape(P, KH, BL)
    return a.transpose(2, 1, 0).reshape(BL, H)


def kernel(tokens, emb, W, U, b):
    tokens = np.ascontiguousarray(np.asarray(tokens), dtype=np.int32)
    emb = np.ascontiguousarray(np.asarray(emb), dtype=np.float32)
    W = np.ascontiguousarray(np.asarray(W), dtype=np.float32)
    U = np.ascontiguousarray(np.asarray(U), dtype=np.float32)
    b = np.ascontiguousarray(np.asarray(b), dtype=np.float32)

    nc = _get_compiled()
    in_maps = []
    for i in range(NCORES):
        in_maps.append(
            {
                "emb": emb,
                "W": W,
                "U": U,
                "bvec": b,
                "tok": make_tok_idx(tokens[i * BL:(i + 1) * BL]),
            }
        )
    res = run_bass_kernel_spmd(nc, in_maps, core_ids=list(range(NCORES))).results

    h = np.zeros((B, H), np.float32)
    c = np.zeros((B, H), np.float32)
    for i in range(NCORES):
        h[i * BL:(i + 1) * BL] = unpack_state(res[i]["ho"])
        c[i * BL:(i + 1) * BL] = unpack_state(res[i]["co"])
    return h, c


def _build_run_fn(nc):
    """jit'd fn running the kernel once on 8 cores (device-resident args)."""
    import jax
    from jax.sharding import Mesh, PartitionSpec
    from jax.experimental.shard_map import shard_map
    import concourse.mybir as mybir_
    from concourse import bass2jax

    bass2jax.install_neuronx_cc_hook()

    partition_name = nc.partition_id_tensor.name if nc.partition_id_tensor else None
    in_names, out_names, out_avals = [], [], []
    for alloc in nc.m.functions[0].allocations:
        if not isinstance(alloc, mybir_.MemoryLocationSet):
            continue
        name = alloc.memorylocations[0].name
        if alloc.kind == "ExternalInput":
            if name != partition_name:
                in_names.append(name)
        elif alloc.kind == "ExternalOutput":
            out_names.append(name)
            out_avals.append(
                jax.core.ShapedArray(
                    tuple(alloc.tensor_shape), mybir_.dt.np(alloc.dtype)
                )
            )
    n_params = len(in_names)
    all_in_names = list(in_names) + list(out_names)
    if partition_name is not None:
        all_in_names.append(partition_name)

    def _body(*args):
        operands = list(args)
        if partition_name is not None:
            operands.append(bass2jax.partition_id_tensor())
        return tuple(
            bass2jax._bass_exec_p.bind(
                *operands,
                out_avals=tuple(out_avals),
                in_names=tuple(all_in_names),
                out_names=tuple(out_names),
                lowering_input_output_aliases=(),
                sim_require_finite=True,
                sim_require_nnan=True,
                nc=nc,
            )
        )

    devices = jax.devices()[:NCORES]
    mesh = Mesh(np.asarray(devices), ("core",))
    nio = n_params + len(out_names)
    fn = jax.jit(
        shard_map(
            _body,
            mesh=mesh,
            in_specs=(PartitionSpec("core"),) * nio,
            out_specs=(PartitionSpec("core"),) * len(out_names),
            check_rep=False,
        )
    )
    return fn, in_names, out_names, out_avals


def _prep_run(nc, in_maps):
    """Build the jitted runner and device-resident args for nc."""
    import jax

    fn, in_names, out_names, out_avals = _build_run_fn(nc)
    concat_in = [
        np.concatenate([in_maps[c][k] for c in range(NCORES)], axis=0)
        for k in in_names
    ]
    concat_zeros = [
        np.zeros((NCORES * a.shape[0], *a.shape[1:]), a.dtype) for a in out_avals
    ]
    args = [jax.device_put(x) for x in concat_in + concat_zeros]
    return fn, args


def _batch_wall(fn, args, batchk):
    """Wall time of `batchk` async-queued device executions (one block)."""
    import time as _time
    import jax

    t0 = _time.perf_counter()
    outs = [fn(*args) for _ in range(batchk)]
    jax.block_until_ready(outs)
    return _time.perf_counter() - t0


def _make_in_maps(np_inputs):
    tokens = np.ascontiguousarray(np.asarray(np_inputs["tokens"]), dtype=np.int32)
    in_maps = []
    for i in range(NCORES):
        in_maps.append(
            {
                "emb": np.asarray(np_inputs["emb"], np.float32),
                "W": np.asarray(np_inputs["W"], np.float32),
                "U": np.asarray(np_inputs["U"], np.float32),
                "bvec": np.asarray(np_inputs["b"], np.float32),
                "tok": make_tok_idx(tokens[i * BL:(i + 1) * BL]),
            }
        )
    return in_maps


def time_kernel_hw(np_inputs, reps_hi=2, calls=8, **build_kw):
    """Estimate one-pass HW time (ns): difference of amplified variants.

    Builds the kernel with the whole compute repeated 1x and reps_hi x,
    times batched device-resident executions of both, and divides the
    delta — dispatch/transfer overhead cancels.
    """
    import jax

    in_maps = _make_in_maps(np_inputs)

    cfg = {**BEST, **build_kw}
    fused = cfg.pop("fused", False)
    variants = {}
    for r in (1, reps_hi):
        nc = bacc.Bacc(None, target_bir_lowering=False)
        build_program(nc, T, 128, reps=r, **cfg)
        if fused:
            fuse_ldweights(nc)
        nc.compile()
        variants[r] = nc

    batchk = 24
    fn1, args1 = _prep_run(variants[1], in_maps)
    fn2, args2 = _prep_run(variants[reps_hi], in_maps)
    # warm both (compile + first exec)
    jax.block_until_ready([fn1(*args1), fn2(*args2)])
    _batch_wall(fn1, args1, 2)
    _batch_wall(fn2, args2, 2)
    # Interleaved R1/R2 batches: dispatch-overhead drift (~ms-scale, slow)
    # cancels in the per-round pairing.
    deltas = []
    for r in range(calls):
        if r % 2 == 0:
            t1 = _batch_wall(fn1, args1, batchk)
            t2 = _batch_wall(fn2, args2, batchk)
        else:
            t2 = _batch_wall(fn2, args2, batchk)
            t1 = _batch_wall(fn1, args1, batchk)
        deltas.append((t2 - t1) / batchk / (reps_hi - 1))
    deltas.sort()
    per_pass_med = deltas[len(deltas) // 2]
    per_pass_mean = sum(deltas) / len(deltas)
    print(
        "timing: per-pass (paired R%d-R1 batches of %d) med %.3f ms, "
        "mean %.3f ms, spread [%.3f, %.3f]"
        % (
            reps_hi,
            batchk,
            per_pass_med * 1e3,
            per_pass_mean * 1e3,
            deltas[0] * 1e3,
            deltas[-1] * 1e3,
        )
    )
    return max(per_pass_med, 0.0) * 1e9




# revision 36
# speedup vs baseline: 1.1515x; 1.1515x over previous
"""
Trainium2 Bass kernel for nn_Encoder (embedding lookup + LSTM, returns final (h, c)).

Strategy (data-parallel over batch, per sharding hint):
  - 8 cores, each handles B_local = 4 of the 32 batch rows.
  - Per core: gather embedding rows via indirect DMA (t-major order),
    transpose on PE, project x @ W with fp16 matmuls (chunked over T),
    then run the 512-step recurrence with U as the stationary operand
    in fp16 producing gates transposed (4H on partitions) so
    activations/cell update run wide on ACT/DVE.
  - Gate layout: psum z tile per pair of H-slices, packed columns
    (gate', b) with gate' order (i, f, o, g) so one sigmoid covers i,f,o
    and one tanh covers g.
  - h is kept as hT [128 x (hs, b)] fp16 which is exactly the moving-operand
    layout the next step's matmuls need.
  - Per-step MM blocks follow ORDER625 (see emit_step_v5): interleaving the
    two psum groups' k-rounds maximizes the slack available to each pair's
    cross-engine activation/update chain.

Performance notes (measured on HW, this session): the recurrence runs at
~64 x ~27ns LDWEIGHTS+MATMUL pairs per step and is pinned there — fp8
weights (e3m4, FWL) change nothing (pace is pair-overhead/weight-load-
floor bound, not weight-byte bound), and folding the LDW into a
self-loading matmul (InstMatmult.ldweights=True) produces NaNs on HW
(walrus needs the standalone LDW for non-fp32). Alternative schedules
(xz injection into PSUM via identity-matmul, 4-way group pipelining,
prep spread into step bubbles) all measure within noise (+-80ns/step).

Host side: shard/marshal inputs, run SPMD on 8 cores, unpack outputs.
"""

import numpy as np

import concourse.bass as bass
import concourse.mybir as mybir
import concourse.tile as tile
from concourse import bacc
from concourse.bass import IndirectOffsetOnAxis
from concourse.bass_utils import run_bass_kernel_spmd
from concourse.masks import make_identity

# Problem constants (hardcoded; harness contract)
B, T, V, E, H = 32, 512, 20000, 300, 512
G4 = 4 * H            # 2048
NCORES = 8
BL = B // NCORES      # 4 batch rows per core
P = 128
KM = G4 // P          # 16 M-tiles over 4H
KH = H // P           # 4 K-tiles over H
KE_SIZES = [128, 128, 44]   # K subtiles over E=300
# Keras gate g (i,f,g,o) -> packed slot (i,f,o,g): sigmoid = slots 0..2, tanh = slot 3
PERM = [0, 1, 3, 2]

f32 = mybir.dt.float32
f32r = mybir.dt.float32r
f16 = mybir.dt.float16
f8e3 = mybir.dt.float8e3
i32 = mybir.dt.int32

AF = mybir.ActivationFunctionType


def build_program(
    nc, T_steps=T, Tc=128, dbg_step=None, reps=1, sched="v2", u8=True, spread=False
):
    """Emit the full per-core program into nc (a bacc.Bacc).

    reps > 1 repeats the whole compute (for timing amplification).

    u8=True stores the recurrent weights U in fp8 (e3m4) scaled by USCALE,
    with the xz path (W, b) carrying the same scale and the activations
    descaling via their input-scale operand. Measured: identical speed to
    fp16 (the per-matmul pace is bound by the LDWEIGHTS+MATMUL pair
    overhead, not weight bytes), ~10x the numeric error — so keep False.

    sched picks the per-step schedule: v2 = pair-contiguous MM order with
    DVE z-add; v3/v4 = xz injected into PSUM via identity-matmul with a
    shortened chain; v5 = v2's chain with an interleaved MM block order
    (ORDER625) that raises both pairs' chain slack from 0.25*S to 0.375*S.
    All measured within noise of each other on hardware (the 512-step
    recurrence is pinned at ~64 LDW+MM pairs x ~27ns/step).

    spread=True (v3/v4/v5) breaks the next chunk's gather/transpose/xz-
    projection into single-op thunks slotted into per-step PE bubbles
    instead of one serial burst."""
    assert T_steps % Tc == 0
    USCALE = 32.0 if u8 else 1.0
    INV = 1.0 / USCALE
    udt = f8e3 if u8 else f16
    nch = T_steps // Tc
    NJ = Tc * BL // P  # gathers (128-row groups) per chunk

    emb_t = nc.declare_dram_parameter("emb", [V, E], f32, isOutput=False)
    W_t = nc.declare_dram_parameter("W", [E, G4], f32, isOutput=False)
    U_t = nc.declare_dram_parameter("U", [H, G4], f32, isOutput=False)
    b_t = nc.declare_dram_parameter("bvec", [G4], f32, isOutput=False)
    tok_t = nc.declare_dram_parameter("tok", [P, T_steps * BL // P], i32, isOutput=False)
    ho_t = nc.declare_dram_parameter("ho", [P, BL * KH], f16, isOutput=True)
    co_t = nc.declare_dram_parameter("co", [P, BL * KH], f32, isOutput=True)
    if dbg_step is not None:
        dbg_z = nc.declare_dram_parameter("dbg_z", [P, 64], f32, isOutput=True)
        dbg_h = nc.declare_dram_parameter("dbg_h", [P, BL * KH], f16, isOutput=True)
        dbg_c = nc.declare_dram_parameter("dbg_c", [P, BL * KH], f32, isOutput=True)

    with tile.TileContext(nc) as tc:
        with (
            tc.tile_pool(name="const", bufs=1) as cpool,
            tc.tile_pool(name="ustage", bufs=2) as upool,
            tc.tile_pool(name="xrows", bufs=4) as xpool,
            tc.tile_pool(name="xtp", bufs=2) as xtpool,
            tc.tile_pool(name="ptr", bufs=2, space="PSUM") as ptr_pool,
            tc.tile_pool(name="pxz", bufs=2, space="PSUM") as pxz_pool,
            tc.tile_pool(name="pz", bufs=4, space="PSUM") as pz_pool,
        ):
            # ---- constants / weights ----
            U16 = cpool.tile([P, KH * G4], udt, tag="U16")
            W_sb = cpool.tile([P, 3 * G4], f16, tag="Wsb")
            b_sb = cpool.tile([P, KM], f32, tag="bsb")
            tok_sb = cpool.tile([P, T_steps * BL // P], i32, tag="tok")
            ident = cpool.tile([P, P], f32, tag="ident")
            h16 = cpool.tile([P, BL * KH], f16, tag="h16")
            cst = cpool.tile([P, BL * KH], f32, tag="cst")
            z_s = cpool.tile([P, 64], f32, tag="zs")
            a_s = cpool.tile([P, 64], f32, tag="as")
            tmp1 = cpool.tile([P, BL * KH], f32, tag="t1")
            tmp2 = cpool.tile([P, BL * KH], f32, tag="t2")
            tct = cpool.tile([P, BL * KH], f32, tag="tct")
            xzdt = f16 if sched in ("v3", "v4") else f32
            xz_sb = [
                cpool.tile([P, Tc * 64], xzdt, tag=f"xz{par}", name=f"xz{par}")
                for par in range(2)
            ]

            make_identity(nc, ident[:])

            # U (fp32 DRAM) -> U16 (fp16 SBUF), K-tile k region at cols k*G4
            for k in range(KH):
                ust = upool.tile([P, G4], f32, tag="ustage")
                nc.sync.dma_start(ust[:], U_t.ap()[k * P:(k + 1) * P, :])
                if u8:
                    nc.vector.tensor_scalar_mul(
                        U16[:, k * G4:(k + 1) * G4], ust[:], USCALE
                    )
                else:
                    nc.vector.tensor_copy(U16[:, k * G4:(k + 1) * G4], ust[:])

            # W: 3 K-subtiles at cols kk*G4, cast to fp16 via staging
            ofs = 0
            for kk, kw in enumerate(KE_SIZES):
                wst = upool.tile([P, G4], f32, tag="ustage", name=f"wst{kk}")
                nc.sync.dma_start(wst[:kw, :], W_t.ap()[ofs:ofs + kw, :])
                if u8:
                    nc.vector.tensor_scalar_mul(
                        W_sb[:kw, kk * G4:(kk + 1) * G4], wst[:kw, :], USCALE
                    )
                else:
                    nc.vector.tensor_copy(
                        W_sb[:kw, kk * G4:(kk + 1) * G4], wst[:kw, :]
                    )
                ofs += kw

            # bias: b_sb[p, m] = b[m*128 + p]
            nc.sync.dma_start(b_sb[:], b_t.ap().rearrange("(m p) -> p m", p=P))
            if u8:
                nc.vector.tensor_scalar_mul(b_sb[:], b_sb[:], USCALE)
            nc.sync.dma_start(tok_sb[:], tok_t.ap())

            nc.gpsimd.memset(h16[:], 0.0)
            nc.gpsimd.memset(cst[:], 0.0)

            def make_prep_thunks(c):
                """Prep for chunk c as a list of single-PE-op thunks.

                thunks[0] issues the gather DMAs (off-PE); the rest each emit
                one PE op (transpose or one xz matmul K-subtile) so they can
                be slotted into per-step PE wait bubbles."""
                xz_dst = xz_sb[c % 2]
                xT = xtpool.tile([P, 3 * Tc * BL], f16, tag="xT", name=f"xT{c}")
                xrs = []
                state = {}

                def gathers():
                    for j in range(NJ):
                        xr = xpool.tile([P, E], f32, tag="xrows", name=f"xr{c}_{j}")
                        xrs.append(xr)
                        nc.gpsimd.indirect_dma_start(
                            out=xr[:],
                            out_offset=None,
                            in_=emb_t.ap(),
                            in_offset=IndirectOffsetOnAxis(
                                ap=tok_sb[:, c * NJ + j:c * NJ + j + 1], axis=0
                            ),
                        )

                thunks = [gathers]

                def tp(j, kk, kw):
                    def run():
                        pt = ptr_pool.tile([P, P], f32, tag="ptr")
                        nc.tensor.transpose(
                            out=pt[:kw, :], in_=xrs[j][:, kk * P:kk * P + kw],
                            identity=ident[:],
                        )
                        nc.vector.tensor_copy(
                            xT[:kw, kk * Tc * BL + j * P:kk * Tc * BL + (j + 1) * P],
                            pt[:kw, :],
                        )
                    return run

                for j in range(NJ):
                    for kk, kw in enumerate(KE_SIZES):
                        thunks.append(tp(j, kk, kw))

                N = Tc * BL

                def mm(m, kk, kw):
                    def run():
                        if kk == 0:
                            state["pxz"] = pxz_pool.tile(
                                [P, N], f32, tag="pxz", name=f"pxz{c}_{m}"
                            )
                        pxz = state["pxz"]
                        nc.tensor.matmul(
                            pxz[:],
                            W_sb[:kw, kk * G4 + m * P:kk * G4 + (m + 1) * P],
                            xT[:kw, kk * N:(kk + 1) * N],
                            start=(kk == 0),
                            stop=(kk == 2),
                        )
                        if kk == 2:
                            # packed dest: col = t*64 + (m%4)*16 + PERM[m//4]*4 + b
                            slot = (m % 4) * 16 + PERM[m // 4] * 4
                            dst = xz_dst[:].rearrange("p (t g) -> p t g", g=64)[
                                :, :, slot:slot + 4
                            ]
                            src = pxz[:].rearrange("p (t b) -> p t b", b=BL)
                            nc.vector.tensor_scalar_add(dst, src, b_sb[:, m:m + 1])
                    return run

                for m in range(KM):
                    for kk, kw in enumerate(KE_SIZES):
                        thunks.append(mm(m, kk, kw))
                return thunks

            def emit_prep(c):
                """Gather + transpose + xz projection for chunk c, all at once."""
                for th in make_prep_thunks(c):
                    th()

            # v3 state/scratch: one tile, 24 cols per hs: [i f o g | c | tct]
            asc = cpool.tile([P, 4 * 24], f32, tag="asc")
            tmp3 = cpool.tile([P, 4 * 8], f32, tag="tmp3")
            ident8 = cpool.tile([P, P], udt, tag="ident8")
            nc.vector.tensor_copy(ident8[:], ident[:])
            nc.gpsimd.memset(asc[:], 0.0)

            # MM emission order for the last K round: group M-tiles by H-slice
            ORDER_LAST = [m for hs in range(4) for m in (hs, 4 + hs, 8 + hs, 12 + hs)]

            def emit_step_v1(c, t):
                psz = [
                    pz_pool.tile([P, 16], f32, tag="pz", name=f"pz{hs}_{c}_{t}")
                    for hs in range(4)
                ]
                for k in range(KH):
                    order = ORDER_LAST if k == KH - 1 else range(KM)
                    for m in order:
                        slot = PERM[m // 4] * 4
                        # start=True marks the whole 2KB psum bank pending-zero,
                        # so only the FIRST matmul touching each psz tile sets it
                        # (round k=0, m in 0..3); later slots overwrite via
                        # pending-zero, later k rounds accumulate.
                        nc.tensor.matmul(
                            psz[m % 4][:, slot:slot + 4],
                            U16[:, k * G4 + m * P:k * G4 + (m + 1) * P],
                            h16[:, k * BL:(k + 1) * BL],
                            start=(k == 0 and m < 4),
                            stop=(k == KH - 1),
                            skip_group_check=True,
                        )
                for hs in range(4):
                    zs = z_s[:, hs * 16:hs * 16 + 16]
                    nc.vector.tensor_add(
                        zs,
                        psz[hs][:],
                        xz_sb[c % 2][:, t * 64 + hs * 16:t * 64 + hs * 16 + 16],
                    )
                    # sigmoid over (i, f, o) slots, tanh over g slot
                    nc.scalar.activation(
                        a_s[:, hs * 16:hs * 16 + 12], z_s[:, hs * 16:hs * 16 + 12],
                        AF.Sigmoid, scale=INV,
                    )
                    nc.scalar.activation(
                        a_s[:, hs * 16 + 12:hs * 16 + 16],
                        z_s[:, hs * 16 + 12:hs * 16 + 16],
                        AF.Tanh, scale=INV,
                    )
                    cs = slice(hs * BL, (hs + 1) * BL)
                    nc.vector.tensor_mul(
                        tmp1[:, cs], a_s[:, hs * 16 + 4:hs * 16 + 8], cst[:, cs]
                    )  # f * c
                    nc.vector.tensor_mul(
                        tmp2[:, cs],
                        a_s[:, hs * 16:hs * 16 + 4],
                        a_s[:, hs * 16 + 12:hs * 16 + 16],
                    )  # i * g
                    nc.vector.tensor_add(cst[:, cs], tmp1[:, cs], tmp2[:, cs])
                    nc.scalar.activation(tct[:, cs], cst[:, cs], AF.Tanh)
                    nc.vector.tensor_mul(
                        h16[:, cs], a_s[:, hs * 16 + 8:hs * 16 + 12], tct[:, cs]
                    )  # h = o * tanh(c), cast to fp16 on write

            def a2(base, width):
                """2D AP over a_s/z_s: [128, (2 hs, width)] at col base within
                each 16-col hs block of the pair being processed."""
                return base.rearrange("p (hs w) -> p hs w", w=16)

            def emit_step_v2(c, t):
                # 2 psum tiles, one per hs-pair; cols = (hs%2)*16 + slot*4 + b
                psz = [
                    pz_pool.tile([P, 32], f32, tag="pz", name=f"pzp{pr}_{c}_{t}")
                    for pr in range(2)
                ]
                # pair-major PE order: all of pair 0's MMs (k-outer), then pair 1
                for pr in range(2):
                    for k in range(KH):
                        for hs in (2 * pr, 2 * pr + 1):
                            for g in range(4):
                                m = g * 4 + hs
                                slot = (hs % 2) * 16 + PERM[g] * 4
                                nc.tensor.matmul(
                                    psz[pr][:, slot:slot + 4],
                                    U16[:, k * G4 + m * P:k * G4 + (m + 1) * P],
                                    h16[:, k * BL:(k + 1) * BL],
                                    start=(k == 0 and hs == 2 * pr and g == 0),
                                    stop=(k == KH - 1),
                                    skip_group_check=True,
                                )
                xz = xz_sb[c % 2]
                for pr in range(2):
                    # per-hs adds (start as soon as that hs' slots are done)
                    for hs in (2 * pr, 2 * pr + 1):
                        nc.vector.tensor_add(
                            z_s[:, hs * 16:hs * 16 + 16],
                            psz[pr][:, (hs % 2) * 16:(hs % 2) * 16 + 16],
                            xz[:, t * 64 + hs * 16:t * 64 + hs * 16 + 16],
                        )
                    h0 = 2 * pr * 16  # base col of this pair in z_s/a_s
                    zs2 = z_s[:].rearrange("p (hs w) -> p hs w", w=16)
                    as2 = a_s[:].rearrange("p (hs w) -> p hs w", w=16)
                    # sigmoid over (i,f,o) of both hs in one 2D-AP instr
                    nc.scalar.activation(
                        as2[:, 2 * pr:2 * pr + 2, 0:12],
                        zs2[:, 2 * pr:2 * pr + 2, 0:12],
                        AF.Sigmoid, scale=INV,
                    )
                    nc.scalar.activation(
                        as2[:, 2 * pr:2 * pr + 2, 12:16],
                        zs2[:, 2 * pr:2 * pr + 2, 12:16],
                        AF.Tanh, scale=INV,
                    )
                    cs = slice(pr * 2 * BL, (pr + 1) * 2 * BL)  # 8 cols of cst
                    c2 = cst[:, cs].rearrange("p (hs b) -> p hs b", b=BL)
                    t1 = tmp1[:, cs].rearrange("p (hs b) -> p hs b", b=BL)
                    t2 = tmp2[:, cs].rearrange("p (hs b) -> p hs b", b=BL)
                    nc.vector.tensor_mul(
                        t1, as2[:, 2 * pr:2 * pr + 2, 4:8], c2
                    )  # f * c
                    nc.vector.tensor_mul(
                        t2,
                        as2[:, 2 * pr:2 * pr + 2, 0:4],
                        as2[:, 2 * pr:2 * pr + 2, 12:16],
                    )  # i * g
                    nc.vector.tensor_add(cst[:, cs], tmp1[:, cs], tmp2[:, cs])
                    nc.scalar.activation(tct[:, cs], cst[:, cs], AF.Tanh)
                    nc.vector.tensor_mul(
                        h16[:, cs].rearrange("p (hs b) -> p hs b", b=BL),
                        as2[:, 2 * pr:2 * pr + 2, 8:12],
                        tct[:, cs].rearrange("p (hs b) -> p hs b", b=BL),
                    )  # h = o * tanh(c), cast to fp16 on write

            def emit_step_v3(c, t, thunk=None):
                """xz injected into PSUM via identity-matmul (no DVE z-add);
                activations read PSUM; per-hs packed state [i f o g | c | tct]
                with one merged mul for [i*g | f*c]. `thunk` (prep work) is
                slotted before pr0's k=2 round, where the PE waits for the
                previous step's pair-1 chain anyway."""
                xz = xz_sb[c % 2]
                psz = [
                    pz_pool.tile([P, 32], f32, tag="pz", name=f"pzv3_{pr}_{c}_{t}")
                    for pr in range(2)
                ]
                for pr in range(2):
                    nc.tensor.matmul(
                        psz[pr][:],
                        ident8[:],
                        xz[:, t * 64 + pr * 32:t * 64 + (pr + 1) * 32],
                        start=True,
                        stop=False,
                        skip_group_check=True,
                    )
                    for k in range(KH):
                        if pr == 0 and k == 2 and thunk is not None:
                            thunk()
                        for hs in (2 * pr, 2 * pr + 1):
                            for g in range(4):
                                m = g * 4 + hs
                                slot = (hs % 2) * 16 + PERM[g] * 4
                                nc.tensor.matmul(
                                    psz[pr][:, slot:slot + 4],
                                    U16[:, k * G4 + m * P:k * G4 + (m + 1) * P],
                                    h16[:, k * BL:(k + 1) * BL],
                                    start=False,
                                    stop=(k == KH - 1),
                                    skip_group_check=True,
                                )
                a3 = asc[:].rearrange("p (hs w) -> p hs w", w=24)
                t3 = tmp3[:].rearrange("p (hs w) -> p hs w", w=8)
                h3 = h16[:].rearrange("p (hs b) -> p hs b", b=BL)
                for pr in range(2):
                    ps3 = psz[pr][:].rearrange("p (hs w) -> p hs w", w=16)
                    hs0 = 2 * pr
                    sl = slice(hs0, hs0 + 2)
                    nc.scalar.activation(
                        a3[:, sl, 0:12], ps3[:, 0:2, 0:12], AF.Sigmoid, scale=INV
                    )
                    nc.scalar.activation(
                        a3[:, sl, 12:16], ps3[:, 0:2, 12:16], AF.Tanh, scale=INV
                    )
                    # [i*g | f*c] in one op: [i,f] x [g,c]
                    nc.vector.tensor_mul(
                        t3[:, sl, :], a3[:, sl, 0:8], a3[:, sl, 12:20]
                    )
                    nc.vector.tensor_add(
                        a3[:, sl, 16:20], t3[:, sl, 0:4], t3[:, sl, 4:8]
                    )
                    nc.scalar.activation(
                        a3[:, sl, 20:24], a3[:, sl, 16:20], AF.Tanh
                    )
                    nc.vector.tensor_mul(
                        h3[:, sl, :], a3[:, sl, 8:12], a3[:, sl, 20:24]
                    )

            # LP-optimal G=2 block order: balances both pairs' chain slack at
            # 0.625*S (vs 0.75*S for pair-contiguous order). (pr, k) blocks.
            ORDER625 = [(0, 0), (0, 1), (1, 0), (0, 2), (0, 3), (1, 1), (1, 2), (1, 3)]

            def emit_step_v4(c, t, thunk=None):
                """v3 chain + interleaved MM block order per ORDER625."""
                xz = xz_sb[c % 2]
                psz = [
                    pz_pool.tile([P, 32], f32, tag="pz", name=f"pzv4_{pr}_{c}_{t}")
                    for pr in range(2)
                ]
                if thunk is not None:
                    thunk()
                started = [False, False]
                for pr, k in ORDER625:
                    if not started[pr]:
                        nc.tensor.matmul(
                            psz[pr][:],
                            ident8[:],
                            xz[:, t * 64 + pr * 32:t * 64 + (pr + 1) * 32],
                            start=True,
                            stop=False,
                            skip_group_check=True,
                        )
                        started[pr] = True
                    for hs in (2 * pr, 2 * pr + 1):
                        for g in range(4):
                            m = g * 4 + hs
                            slot = (hs % 2) * 16 + PERM[g] * 4
                            nc.tensor.matmul(
                                psz[pr][:, slot:slot + 4],
                                U16[:, k * G4 + m * P:k * G4 + (m + 1) * P],
                                h16[:, k * BL:(k + 1) * BL],
                                start=False,
                                stop=(k == KH - 1),
                                skip_group_check=True,
                            )
                a3 = asc[:].rearrange("p (hs w) -> p hs w", w=24)
                t3 = tmp3[:].rearrange("p (hs w) -> p hs w", w=8)
                h3 = h16[:].rearrange("p (hs b) -> p hs b", b=BL)
                for pr in range(2):
                    ps3 = psz[pr][:].rearrange("p (hs w) -> p hs w", w=16)
                    hs0 = 2 * pr
                    sl = slice(hs0, hs0 + 2)
                    nc.scalar.activation(
                        a3[:, sl, 0:12], ps3[:, 0:2, 0:12], AF.Sigmoid, scale=INV
                    )
                    nc.scalar.activation(
                        a3[:, sl, 12:16], ps3[:, 0:2, 12:16], AF.Tanh, scale=INV
                    )
                    nc.vector.tensor_mul(
                        t3[:, sl, :], a3[:, sl, 0:8], a3[:, sl, 12:20]
                    )
                    nc.vector.tensor_add(
                        a3[:, sl, 16:20], t3[:, sl, 0:4], t3[:, sl, 4:8]
                    )
                    nc.scalar.activation(
                        a3[:, sl, 20:24], a3[:, sl, 16:20], AF.Tanh
                    )
                    nc.vector.tensor_mul(
                        h3[:, sl, :], a3[:, sl, 8:12], a3[:, sl, 20:24]
                    )

            def emit_step_v5(c, t, thunk=None):
                """v2's chain (DVE z-add, no identity-MM) with the ORDER625
                interleaved MM block order: both pairs' chain slack becomes
                0.625*S (vs 0.75*S pair-contiguous), hiding the chain fully."""
                psz = [
                    pz_pool.tile([P, 32], f32, tag="pz", name=f"pzv5_{pr}_{c}_{t}")
                    for pr in range(2)
                ]
                started = [False, False]
                if thunk is not None:
                    thunk()
                for pr, k in ORDER625:
                    first = not started[pr]
                    started[pr] = True
                    for hs in (2 * pr, 2 * pr + 1):
                        for g in range(4):
                            m = g * 4 + hs
                            slot = (hs % 2) * 16 + PERM[g] * 4
                            nc.tensor.matmul(
                                psz[pr][:, slot:slot + 4],
                                U16[:, k * G4 + m * P:k * G4 + (m + 1) * P],
                                h16[:, k * BL:(k + 1) * BL],
                                start=(first and hs == 2 * pr and g == 0),
                                stop=(k == KH - 1),
                                skip_group_check=True,
                            )
                xz = xz_sb[c % 2]
                for pr in range(2):
                    for hs in (2 * pr, 2 * pr + 1):
                        nc.vector.tensor_add(
                            z_s[:, hs * 16:hs * 16 + 16],
                            psz[pr][:, (hs % 2) * 16:(hs % 2) * 16 + 16],
                            xz[:, t * 64 + hs * 16:t * 64 + hs * 16 + 16],
                        )
                    zs2 = z_s[:].rearrange("p (hs w) -> p hs w", w=16)
                    as2 = a_s[:].rearrange("p (hs w) -> p hs w", w=16)
                    nc.scalar.activation(
                        as2[:, 2 * pr:2 * pr + 2, 0:12],
                        zs2[:, 2 * pr:2 * pr + 2, 0:12],
                        AF.Sigmoid, scale=INV,
                    )
                    nc.scalar.activation(
                        as2[:, 2 * pr:2 * pr + 2, 12:16],
                        zs2[:, 2 * pr:2 * pr + 2, 12:16],
                        AF.Tanh, scale=INV,
                    )
                    cs = slice(pr * 2 * BL, (pr + 1) * 2 * BL)
                    c2 = cst[:, cs].rearrange("p (hs b) -> p hs b", b=BL)
                    t1 = tmp1[:, cs].rearrange("p (hs b) -> p hs b", b=BL)
                    t2 = tmp2[:, cs].rearrange("p (hs b) -> p hs b", b=BL)
                    nc.vector.tensor_mul(
                        t1, as2[:, 2 * pr:2 * pr + 2, 4:8], c2
                    )
                    nc.vector.tensor_mul(
                        t2,
                        as2[:, 2 * pr:2 * pr + 2, 0:4],
                        as2[:, 2 * pr:2 * pr + 2, 12:16],
                    )
                    nc.vector.tensor_add(cst[:, cs], tmp1[:, cs], tmp2[:, cs])
                    nc.scalar.activation(tct[:, cs], cst[:, cs], AF.Tanh)
                    nc.vector.tensor_mul(
                        h16[:, cs].rearrange("p (hs b) -> p hs b", b=BL),
                        as2[:, 2 * pr:2 * pr + 2, 8:12],
                        tct[:, cs].rearrange("p (hs b) -> p hs b", b=BL),
                    )

            emit_step = {
                "v1": emit_step_v1,
                "v2": emit_step_v2,
                "v3": emit_step_v3,
                "v4": emit_step_v4,
                "v5": emit_step_v5,
            }[sched]

            for rep in range(reps):
                if rep > 0:
                    nc.gpsimd.memset(h16[:], 0.0)
                    nc.gpsimd.memset(cst[:], 0.0)
                    nc.gpsimd.memset(asc[:], 0.0)
                emit_prep(0)
                pending = []
                for c in range(nch):
                    for t in range(Tc):
                        thunk = None
                        if spread and sched in ("v3", "v4", "v5") and pending and t >= 16:
                            thunk = pending.pop(0)
                        emit_step(c, t, thunk=thunk) if sched in (
                            "v3", "v4", "v5"
                        ) else emit_step(c, t)
                        if dbg_step is not None and (c, t) == dbg_step:
                            nc.sync.dma_start(dbg_z.ap(), z_s[:])
                            nc.sync.dma_start(dbg_h.ap(), h16[:])
                            nc.sync.dma_start(dbg_c.ap(), cst[:])
                        if t == 8 and c + 1 < nch:
                            if spread and sched in ("v3", "v4", "v5"):
                                ths = make_prep_thunks(c + 1)
                                ths[0]()  # gathers go out immediately (off-PE)
                                pending = ths[1:]
                            else:
                                emit_prep(c + 1)
                    assert not pending, f"{len(pending)} prep thunks left"

            nc.sync.dma_start(ho_t.ap(), h16[:])
            if sched in ("v3", "v4"):
                nc.sync.dma_start(
                    co_t.ap(),
                    asc[:].rearrange("p (hs w) -> p hs w", w=24)[:, :, 16:20],
                )
            else:
                nc.sync.dma_start(co_t.ap(), cst[:])

    return nc


def fuse_ldweights(nc):
    """Fold each standalone InstLdweights into its paired InstMatmult
    (self-loading matmul), halving the PE instruction count. bass emits
    LDW directly before its matmul; the LDW carries the weights-tile
    dependency, which must move onto the matmul."""
    n = 0
    for f in nc.m.functions:
        for blk in f.blocks:
            pending = None
            keep = []
            for ins in blk.instructions:
                if isinstance(ins, mybir.InstLdweights):
                    assert pending is None, "two LDWs without a matmul between"
                    pending = ins
                    continue
                if (
                    isinstance(ins, mybir.InstMatmult)
                    and not (ins.is_transpose or False)
                    and pending is not None
                ):
                    ins.ldweights = True
                    ins.merge_dependencies_from(pending)
                    pending = None
                    n += 1
                keep.append(ins)
            assert pending is None, "trailing LDW without matmul"
            blk.instructions = keep
    # move_matmul_waits_to_ldweights scans backward for an LDW per multi-wait
    # matmul; with no LDWs left it degenerates to an O(n^2) full-block scan
    # (~20min at T=512) and has nothing to do anyway — skip it.
    nc.move_matmul_waits_to_ldweights = lambda: None
    return n


_CACHE = {}


# Best verified configuration for the shipped kernel() path.
# v5 = v2's chain with the LP-optimal interleaved MM order (never more
# stall than v2 at equal instruction count; measured equal-or-slightly
# faster, bit-identical numerics).
BEST = dict(sched="v5", u8=False, spread=False, fused=False)


def _get_compiled(T_steps=T, Tc=128, **kw):
    cfg = {**BEST, **kw}
    key = (T_steps, Tc, tuple(sorted(cfg.items())))
    if key not in _CACHE:
        fused = cfg.pop("fused")
        nc = bacc.Bacc(None, target_bir_lowering=False)
        build_program(nc, T_steps, Tc, **cfg)
        if fused:
            fuse_ldweights(nc)
        nc.compile()
        _CACHE[key] = nc
    return _CACHE[key]


def make_tok_idx(tokens_slice, T_steps=T):
    """tokens_slice [BL, T] -> [128, T*BL/128] int32, [p, j] = t-major flat[j*128+p]."""
    flat = tokens_slice.T.reshape(-1)  # index n = t*BL + b
    return np.ascontiguousarray(
        flat.reshape(T_steps * BL // P, P).T.astype(np.int32)
    )


def unpack_state(arr):
    """[128, 16] packed (p, hs*4+b) -> [BL, H]."""
    a = np.asarray(arr).astype(np.float32).reshape(P, KH, BL)
    return a.transpose(2, 1, 0).reshape(BL, H)


def kernel(tokens, emb, W, U, b):
    tokens = np.ascontiguousarray(np.asarray(tokens), dtype=np.int32)
    emb = np.ascontiguousarray(np.asarray(emb), dtype=np.float32)
    W = np.ascontiguousarray(np.asarray(W), dtype=np.float32)
    U = np.ascontiguousarray(np.asarray(U), dtype=np.float32)
    b = np.ascontiguousarray(np.asarray(b), dtype=np.float32)

    nc = _get_compiled()
    in_maps = []
    for i in range(NCORES):
        in_maps.append(
            {
                "emb": emb,
                "W": W,
                "U": U,
                "bvec": b,
                "tok": make_tok_idx(tokens[i * BL:(i + 1) * BL]),
            }
        )
    res = run_bass_kernel_spmd(nc, in_maps, core_ids=list(range(NCORES))).results

    h = np.zeros((B, H), np.float32)
    c = np.zeros((B, H), np.float32)
    for i in range(NCORES):
        h[i * BL:(i + 1) * BL] = unpack_state(res[i]["ho"])
        c[i * BL:(i + 1) * BL] = unpack_state(res[i]["co"])
    return h, c


def _build_run_fn(nc):
    """jit'd fn running the kernel once on 8 cores (device-resident args)."""
    import jax
    from jax.sharding import Mesh, PartitionSpec
    from jax.experimental.shard_map import shard_map
    import concourse.mybir as mybir_
    from concourse import bass2jax

    bass2jax.install_neuronx_cc_hook()

    partition_name = nc.partition_id_tensor.name if nc.partition_id_tensor else None
    in_names, out_names, out_avals = [], [], []
    for alloc in nc.m.functions[0].allocations:
        if not isinstance(alloc, mybir_.MemoryLocationSet):
            continue
        name = alloc.memorylocations[0].name
        if alloc.kind == "ExternalInput":
            if name != partition_name:
                in_names.append(name)
        elif alloc.kind == "ExternalOutput":
            out_names.append(name)
            out_avals.append(
                jax.core.ShapedArray(
                    tuple(alloc.tensor_shape), mybir_.dt.np(alloc.dtype)
                )
            )
    n_params = len(in_names)
    all_in_names = list(in_names) + list(out_names)
    if partition_name is not None:
        all_in_names.append(partition_name)

    def _body(*args):
        operands = list(args)
        if partition_name is not None:
            operands.append(bass2jax.partition_id_tensor())
        return tuple(
            bass2jax._bass_exec_p.bind(
                *operands,
                out_avals=tuple(out_avals),
                in_names=tuple(all_in_names),
                out_names=tuple(out_names),
                lowering_input_output_aliases=(),
                sim_require_finite=True,
                sim_require_nnan=True,
                nc=nc,
            )
        )

    devices = jax.devices()[:NCORES]
    mesh = Mesh(np.asarray(devices), ("core",))
    nio = n_params + len(out_names)
    fn = jax.jit(
        shard_map(
            _body,
            mesh=mesh,
            in_specs=(PartitionSpec("core"),) * nio,
            out_specs=(PartitionSpec("core"),) * len(out_names),
            check_rep=False,
        )
    )
    return fn, in_names, out_names, out_avals


def _prep_run(nc, in_maps):
    """Build the jitted runner and device-resident args for nc."""
    import jax

    fn, in_names, out_names, out_avals = _build_run_fn(nc)
    concat_in = [
        np.concatenate([in_maps[c][k] for c in range(NCORES)], axis=0)
        for k in in_names
    ]
    concat_zeros = [
        np.zeros((NCORES * a.shape[0], *a.shape[1:]), a.dtype) for a in out_avals
    ]
    args = [jax.device_put(x) for x in concat_in + concat_zeros]
    return fn, args


def _batch_wall(fn, args, batchk):
    """Wall time of `batchk` async-queued device executions (one block)."""
    import time as _time
    import jax

    t0 = _time.perf_counter()
    outs = [fn(*args) for _ in range(batchk)]
    jax.block_until_ready(outs)
    return _time.perf_counter() - t0


def _make_in_maps(np_inputs):
    tokens = np.ascontiguousarray(np.asarray(np_inputs["tokens"]), dtype=np.int32)
    in_maps = []
    for i in range(NCORES):
        in_maps.append(
            {
                "emb": np.asarray(np_inputs["emb"], np.float32),
                "W": np.asarray(np_inputs["W"], np.float32),
                "U": np.asarray(np_inputs["U"], np.float32),
                "bvec": np.asarray(np_inputs["b"], np.float32),
                "tok": make_tok_idx(tokens[i * BL:(i + 1) * BL]),
            }
        )
    return in_maps


def time_kernel_hw(np_inputs, reps_hi=2, calls=10, **build_kw):
    """Estimate one-pass HW time (ns): difference of amplified variants.

    Builds the kernel with the whole compute repeated 1x and reps_hi x,
    times batched device-resident executions of both, and divides the
    delta — dispatch/transfer overhead cancels.
    """
    import jax

    in_maps = _make_in_maps(np_inputs)

    cfg = {**BEST, **build_kw}
    fused = cfg.pop("fused", False)
    variants = {}
    for r in (1, reps_hi):
        nc = bacc.Bacc(None, target_bir_lowering=False)
        build_program(nc, T, 128, reps=r, **cfg)
        if fused:
            fuse_ldweights(nc)
        nc.compile()
        variants[r] = nc

    batchk = 48
    fn1, args1 = _prep_run(variants[1], in_maps)
    fn2, args2 = _prep_run(variants[reps_hi], in_maps)
    # warm both (compile + first exec)
    jax.block_until_ready([fn1(*args1), fn2(*args2)])
    _batch_wall(fn1, args1, 2)
    _batch_wall(fn2, args2, 2)
    # Interleaved R1/R2 batches: dispatch-overhead drift (~ms-scale, slow)
    # cancels in the per-round pairing.
    deltas = []
    for r in range(calls):
        if r % 2 == 0:
            t1 = _batch_wall(fn1, args1, batchk)
            t2 = _batch_wall(fn2, args2, batchk)
        else:
            t2 = _batch_wall(fn2, args2, batchk)
            t1 = _batch_wall(fn1, args1, batchk)
        deltas.append((t2 - t1) / batchk / (reps_hi - 1))
    deltas.sort()
    per_pass_med = deltas[len(deltas) // 2]
    per_pass_mean = sum(deltas) / len(deltas)
    print(
        "timing: per-pass (paired R%d-R1 batches of %d) med %.3f ms, "
        "mean %.3f ms, spread [%.3f, %.3f]"
        % (
            reps_hi,
            batchk,
            per_pass_med * 1e3,
            per_pass_mean * 1e3,
            deltas[0] * 1e3,
            deltas[-1] * 1e3,
        )
    )
    return max(per_pass_med, 0.0) * 1e9


